# revision 1
# baseline (speedup 1.0000x reference)
"""DeepseekV3 decoder layer (MLA + SwiGLU MLP), T=2048 prefill, fp32 I/O.

Sharding: sequence-parallel striped — core c owns token rows c::8 (256 rows),
so all 8 cores run one identical SPMD program with balanced causal work; only
input data differs per core. The KV latent path (all 2048 tokens) is
replicated on every core; outputs are disjoint row sets concatenated on host.

Per core: row-major activations (per-token norm scales are per-partition),
bf16 matmul operands with fp32 PSUM accumulation, LN weights folded into
adjacent GEMMs on host, RoPE via host cos/sin tables, softmax without
max-subtraction (scores are O(30) max), denominator via ones-matmul,
causal masking by static tile skipping + mask multiply. The q_b/absorb
path runs lazily per head inside the attention loop to bound SBUF.
"""

import numpy as np
import ml_dtypes

bfloat16 = ml_dtypes.bfloat16

T = 2048
H = 2048
NH = 16
QLR = 1536
KVLR = 512
DN = 128
DR = 64
DV = 128
INTER = 10944
NCORES = 8
RPC = T // NCORES
NQT = RPC // 128
NTT = T // 128
NFC = H // 128
NRC = QLR // 128
NKV = KVLR // 128
NIT = 86
IPAD = NIT * 128
EPS = 1e-6
SCALE = (DN + DR) ** -0.5
THETA = 10000.0
QH = DN + DR               # 192 per-head q dim

_CACHE = {}


def _build_module():
    import os
    MAXPH = int(os.environ.get("KERNEL_MAXPH", "9"))
    import concourse.bass as bass
    import concourse.tile as tile
    from concourse import bacc, mybir

    f32 = mybir.dt.float32
    bf16 = mybir.dt.bfloat16
    AF = mybir.ActivationFunctionType
    ALU = mybir.AluOpType

    nc = bacc.Bacc("TRN2", target_bir_lowering=False, debug=False,
                   enable_asserts=False, num_devices=NCORES)

    def inp(name, shape, dt):
        return nc.dram_tensor(name, list(shape), dt, kind="ExternalInput").ap()

    # per-core inputs
    x_rows = inp("x_rows", [NQT, 128, H], f32)
    xTc = inp("xTc", [NFC, 128, RPC], bf16)
    cosq = inp("cosq", [NQT, 128, DR // 2], f32)
    sinq = inp("sinq", [NQT, 128, DR // 2], f32)
    masks = inp("masks", [NTT, 128, RPC], bf16)
    # replicated inputs
    xstat = inp("xstat", [NTT, 128, H], bf16)
    xT_blk = inp("xT_blk", [NTT, 128, NFC, 128], bf16)
    qa_blk = inp("qa_blk", [NFC, 128, QLR], bf16)
    qb_blk = inp("qb_blk", [NH, NRC, 128, QH], bf16)
    kva_blk = inp("kva_blk", [NFC, 128, KVLR + DR], bf16)
    wuk = inp("wuk", [NH, 128, NKV, 128], bf16)
    wuv = inp("wuv", [NH, 128, NKV, DV], bf16)
    ow_blk = inp("ow_blk", [NH, 128, H], bf16)
    gu_blk = inp("gu_blk", [2, NIT, 128, NFC, 128], bf16)
    dw_blk = inp("dw_blk", [NIT, 128, H], bf16)
    cosk = inp("cosk", [128, NTT, DR // 2], f32)
    sink = inp("sink", [128, NTT, DR // 2], f32)
    eye = inp("eye", [128, 128], bf16)
    ones = inp("ones", [128, 1], bf16)

    out_rows = nc.dram_tensor("out_rows", [NQT, 128, H], f32,
                              kind="ExternalOutput").ap()

    from contextlib import ExitStack
    with tile.TileContext(nc) as tc, ExitStack() as ctx:
        persist = ctx.enter_context(tc.tile_pool(name="persist", bufs=1))

        def pt(shape, dt, tag):
            return persist.tile(list(shape), dt, tag=tag, name=tag)

        eps_sb = pt([128, 1], f32, "eps")
        nc.vector.memset(eps_sb[:], EPS)
        eye_sb = pt([128, 128], bf16, "eye")
        nc.sync.dma_start(out=eye_sb[:], in_=eye[:])
        ones_sb = pt([128, 1], bf16, "ones")
        nc.sync.dma_start(out=ones_sb[:], in_=ones[:])
        x_rows_sb = pt([128, NQT, H], f32, "x_rows")
        for qt in range(NQT):
            nc.sync.dma_start(out=x_rows_sb[:, qt, :], in_=x_rows[qt])

        rstd_all = pt([128, NTT], f32, "rstd_all")
        s_ck = pt([128, NTT], f32, "s_ck")
        c_hat = pt([128, NTT, KVLR], bf16, "c_hat")
        kT_lat = pt([128, NKV, T], bf16, "kT_lat")
        kT_rope = pt([64, T], bf16, "kT_rope")
        qcT = pt([128, NRC, RPC], bf16, "qcT")
        o_vT = pt([128, NH, RPC], bf16, "o_vT")
        hnT = pt([128, NFC, RPC], bf16, "hnT")
        act_all = pt([128, NIT, RPC], bf16, "act_all")

        # =================== phase 0: stats + kv path ===================
        with tc.tile_pool(name="p0", bufs=3) as p0, \
             tc.tile_pool(name="p0w", bufs=NFC) as p0w, \
             tc.tile_pool(name="p0s", bufs=1) as p0s, \
             tc.tile_pool(name="p0d", bufs=2) as p0d, \
             tc.tile_pool(name="p0ps", bufs=2, space="PSUM") as p0ps, \
             tc.tile_pool(name="p0tp", bufs=2, space="PSUM") as p0tp:
            cosk_sb = p0s.tile([128, NTT, DR // 2], f32, name="cosk_sb")
            nc.sync.dma_start(out=cosk_sb[:], in_=cosk[:])
            sink_sb = p0s.tile([128, NTT, DR // 2], f32, name="sink_sb")
            nc.sync.dma_start(out=sink_sb[:], in_=sink[:])
            ssq_all = p0s.tile([128, NTT], f32, name="ssq_all")
            ssq_kv = p0s.tile([128, NTT], f32, name="ssq_kv")
            c_raw = p0s.tile([128, NTT, KVLR + DR], bf16, name="c_raw")
            for tt in range(NTT):
                xs = p0.tile([128, H], bf16, tag="xs", name="xs")
                nc.gpsimd.dma_start(out=xs[:], in_=xstat[tt])
                scrap = p0d.tile([128, H], bf16, tag="scrap", name="scrap")
                nc.vector.scalar_tensor_tensor(
                    scrap[:], xs[:], 1.0, xs[:], ALU.bypass, ALU.mult,
                    accum_out=ssq_all[:, tt:tt + 1])
            nc.scalar.activation(rstd_all[:], ssq_all[:], AF.Ln,
                                 bias=eps_sb[:], scale=1.0 / H)
            nc.scalar.activation(rstd_all[:], rstd_all[:], AF.Exp, scale=-0.5)

            kvw = []
            for fc in range(NFC):
                w = p0w.tile([128, KVLR + DR], bf16, tag="kvw", name="kvw")
                nc.sync.dma_start(out=w[:], in_=kva_blk[fc])
                kvw.append(w)
            for tt in range(NTT):
                xt = p0.tile([128, NFC, 128], bf16, tag="xt", name="xt")
                nc.gpsimd.dma_start(out=xt[:], in_=xT_blk[tt])
                ps = p0ps.tile([128, KVLR + DR], f32, tag="kvps", name="kvps")
                for fc in range(NFC):
                    nc.tensor.matmul(ps[:, 0:512], xt[:, fc, :],
                                     kvw[fc][:, 0:512],
                                     start=(fc == 0), stop=(fc == NFC - 1))
                    nc.tensor.matmul(ps[:, 512:576], xt[:, fc, :],
                                     kvw[fc][:, 512:576],
                                     start=(fc == 0), stop=(fc == NFC - 1))
                scr2 = p0d.tile([128, KVLR], bf16, tag="scr2", name="scr2")
                nc.scalar.activation(scr2[:], ps[:, 0:512], AF.Square,
                                     accum_out=ssq_kv[:, tt:tt + 1])
                nc.vector.tensor_copy(c_raw[:, tt, :], ps[:])
            t1 = p0s.tile([128, NTT], f32, name="t1")
            nc.vector.tensor_mul(t1[:], rstd_all[:], rstd_all[:])
            nc.vector.tensor_mul(t1[:], t1[:], ssq_kv[:])
            nc.scalar.activation(t1[:], t1[:], AF.Ln, bias=eps_sb[:],
                                 scale=1.0 / KVLR)
            nc.scalar.activation(t1[:], t1[:], AF.Exp, scale=-0.5)
            nc.vector.tensor_mul(s_ck[:], rstd_all[:], t1[:])
            for tt in range(NTT):
                nc.vector.tensor_scalar_mul(c_hat[:, tt, :], c_raw[:, tt, 0:512],
                                            s_ck[:, tt:tt + 1])
            kr = p0s.tile([128, NTT, DR], bf16, name="kr")
            krf = p0s.tile([128, NTT, DR], bf16, name="krf")
            for tt in range(NTT):
                nc.vector.tensor_scalar_mul(kr[:, tt, :], c_raw[:, tt, 512:576],
                                            rstd_all[:, tt:tt + 1])
            x1 = kr[:, :, 0:DR:2]
            x2 = kr[:, :, 1:DR:2]
            ta = p0s.tile([128, NTT, DR // 2], f32, name="ta")
            tb = p0s.tile([128, NTT, DR // 2], f32, name="tb")
            nc.vector.tensor_mul(ta[:], x1, cosk_sb[:])
            nc.vector.tensor_mul(tb[:], x2, sink_sb[:])
            nc.vector.tensor_sub(krf[:, :, 0:DR:2], ta[:], tb[:])
            nc.vector.tensor_mul(ta[:], x2, cosk_sb[:])
            nc.vector.tensor_mul(tb[:], x1, sink_sb[:])
            nc.vector.tensor_add(krf[:, :, 1:DR:2], ta[:], tb[:])
            for tt in range(NTT):
                for rc in range(NKV):
                    tp = p0tp.tile([128, 128], bf16, tag="tp", name="tp")
                    nc.tensor.transpose(tp[:],
                                        c_hat[:, tt, rc * 128:(rc + 1) * 128],
                                        eye_sb[:])
                    nc.any.tensor_copy(kT_lat[:, rc, tt * 128:(tt + 1) * 128],
                                       tp[:])
                tp = p0tp.tile([128, 128], bf16, tag="tp", name="tp")
                nc.tensor.transpose(tp[0:64, :], krf[:, tt, :], eye_sb[:])
                nc.any.tensor_copy(kT_rope[:, tt * 128:(tt + 1) * 128],
                                   tp[0:64, :])


        # =================== phase 1: q_a -> qcT ===================
        if MAXPH >= 1:
            with tc.tile_pool(name="p1", bufs=3) as p1, \
                 tc.tile_pool(name="p1s", bufs=1) as p1s, \
                 tc.tile_pool(name="p1d", bufs=2) as p1d, \
                 tc.tile_pool(name="p1ps", bufs=2, space="PSUM") as p1ps, \
                 tc.tile_pool(name="p1tp", bufs=2, space="PSUM") as p1tp:
                rstd_rows = p1s.tile([128, NQT], f32, name="rstd_rows")
                ssq_r = p1s.tile([128, NQT], f32, name="ssq_r")
                for qt in range(NQT):
                    scrap = p1d.tile([128, H], bf16, tag="scrapq", name="scrapq")
                    nc.vector.scalar_tensor_tensor(
                        scrap[:], x_rows_sb[:, qt, :], 1.0, x_rows_sb[:, qt, :],
                        ALU.bypass, ALU.mult, accum_out=ssq_r[:, qt:qt + 1])
                nc.scalar.activation(rstd_rows[:], ssq_r[:], AF.Ln,
                                     bias=eps_sb[:], scale=1.0 / H)
                nc.scalar.activation(rstd_rows[:], rstd_rows[:], AF.Exp, scale=-0.5)

                xTc_sb = p1s.tile([128, NFC, RPC], bf16, name="xTc_sb")
                for fc in range(NFC):
                    nc.sync.dma_start(out=xTc_sb[:, fc, :], in_=xTc[fc])
                qa_ps = [p1ps.tile([128, QLR], f32, tag="mm", name="mm")
                         for _ in range(NQT)]
                for fc in range(NFC):
                    qaw = p1.tile([128, QLR], bf16, tag="qaw", name="qaw")
                    nc.sync.dma_start(out=qaw[:], in_=qa_blk[fc])
                    for qt in range(NQT):
                        for nn in range(QLR // 512):
                            nc.tensor.matmul(
                                qa_ps[qt][:, nn * 512:(nn + 1) * 512],
                                xTc_sb[:, fc, qt * 128:(qt + 1) * 128],
                                qaw[:, nn * 512:(nn + 1) * 512],
                                start=(fc == 0), stop=(fc == NFC - 1))
                qc = p1s.tile([128, NQT, QLR], bf16, name="qc")
                ssq_q = p1s.tile([128, NQT], f32, name="ssq_q")
                for qt in range(NQT):
                    scr = p1d.tile([128, QLR], bf16, tag="scrq2", name="scrq2")
                    nc.scalar.activation(scr[:], qa_ps[qt][:], AF.Square,
                                         accum_out=ssq_q[:, qt:qt + 1])
                sq = p1s.tile([128, NQT], f32, name="sq")
                nc.vector.tensor_mul(sq[:], rstd_rows[:], rstd_rows[:])
                nc.vector.tensor_mul(sq[:], sq[:], ssq_q[:])
                nc.scalar.activation(sq[:], sq[:], AF.Ln, bias=eps_sb[:],
                                     scale=1.0 / QLR)
                nc.scalar.activation(sq[:], sq[:], AF.Exp, scale=-0.5)
                nc.vector.tensor_mul(sq[:], rstd_rows[:], sq[:])
                for qt in range(NQT):
                    nc.vector.tensor_scalar_mul(qc[:, qt, :], qa_ps[qt][:],
                                                sq[:, qt:qt + 1])
                for qt in range(NQT):
                    for rc in range(NRC):
                        tp = p1tp.tile([128, 128], bf16, tag="tp", name="tp")
                        nc.tensor.transpose(tp[:], qc[:, qt, rc * 128:(rc + 1) * 128],
                                            eye_sb[:])
                        nc.any.tensor_copy(qcT[:, rc, qt * 128:(qt + 1) * 128],
                                           tp[:])


        # =================== phase 2: per-head q_b + attention ================
        if MAXPH >= 2:
            with tc.tile_pool(name="p2", bufs=2) as p2, \
                 tc.tile_pool(name="p2s", bufs=1) as p2s, \
                 tc.tile_pool(name="p2d", bufs=2) as p2d, \
                 tc.tile_pool(name="p2ps", bufs=2, space="PSUM") as p2ps, \
                 tc.tile_pool(name="p2tp", bufs=2, space="PSUM") as p2tp, \
                 tc.tile_pool(name="olps", bufs=2, space="PSUM") as olps, \
                 tc.tile_pool(name="denps", bufs=2, space="PSUM") as denps:
                masks_sb = p2s.tile([128, NTT, RPC], bf16, name="masks_sb")
                for kt in range(NTT):
                    nc.sync.dma_start(out=masks_sb[:, kt, :], in_=masks[kt])
                cosq_sb = p2s.tile([128, NQT, DR // 2], f32, name="cosq_sb")
                sinq_sb = p2s.tile([128, NQT, DR // 2], f32, name="sinq_sb")
                for qt in range(NQT):
                    nc.sync.dma_start(out=cosq_sb[:, qt, :], in_=cosq[qt])
                    nc.sync.dma_start(out=sinq_sb[:, qt, :], in_=sinq[qt])

                for h in range(NH):
                    qbw = p2.tile([128, NRC, QH], bf16, tag="qbw", name="qbw")
                    for rc in range(NRC):
                        nc.sync.dma_start(out=qbw[:, rc, :], in_=qb_blk[h, rc])
                    wuk_h = p2.tile([128, NKV, 128], bf16, tag="wuk_h", name="wuk_h")
                    nc.sync.dma_start(out=wuk_h[:], in_=wuk[h])
                    wuv_h = p2.tile([128, NKV, DV], bf16, tag="wuv_h", name="wuv_h")
                    nc.sync.dma_start(out=wuv_h[:], in_=wuv[h])

                    qb_ps = [p2ps.tile([128, QH], f32, tag="sp", name="qbps")
                             for _ in range(NQT)]
                    for rc in range(NRC):
                        for qt in range(NQT):
                            nc.tensor.matmul(
                                qb_ps[qt][:], qcT[:, rc, qt * 128:(qt + 1) * 128],
                                qbw[:, rc, :], start=(rc == 0),
                                stop=(rc == NRC - 1))
                    q_nope = p2d.tile([128, NQT, DN], bf16, tag="q_nope",
                                      name="q_nope")
                    q_rope = p2d.tile([128, NQT, DR], bf16, tag="q_rope",
                                      name="q_rope")
                    for qt in range(NQT):
                        nc.scalar.activation(q_nope[:, qt, :], qb_ps[qt][:, 0:DN],
                                             AF.Copy, scale=SCALE)
                        nc.scalar.activation(q_rope[:, qt, :], qb_ps[qt][:, DN:],
                                             AF.Copy, scale=SCALE)
                        x1 = q_rope[:, qt, 0:DR:2]
                        x2 = q_rope[:, qt, 1:DR:2]
                        ta = p2d.tile([128, DR // 2], f32, tag="taq", name="taq")
                        tb = p2d.tile([128, DR // 2], f32, tag="tbq", name="tbq")
                        rs1 = p2d.tile([128, DR // 2], bf16, tag="rs1", name="rs1")
                        rs2 = p2d.tile([128, DR // 2], bf16, tag="rs2", name="rs2")
                        nc.vector.tensor_mul(ta[:], x1, cosq_sb[:, qt, :])
                        nc.vector.tensor_mul(tb[:], x2, sinq_sb[:, qt, :])
                        nc.vector.tensor_sub(rs1[:], ta[:], tb[:])
                        nc.vector.tensor_mul(ta[:], x2, cosq_sb[:, qt, :])
                        nc.vector.tensor_mul(tb[:], x1, sinq_sb[:, qt, :])
                        nc.vector.tensor_add(rs2[:], ta[:], tb[:])
                        nc.vector.tensor_copy(q_rope[:, qt, 0:DR:2], rs1[:])
                        nc.vector.tensor_copy(q_rope[:, qt, 1:DR:2], rs2[:])
                    qT = p2d.tile([128, 5, RPC], bf16, tag="qT", name="qT")
                    qnT = p2d.tile([128, NQT, 128], bf16, tag="qnT", name="qnT")
                    for qt in range(NQT):
                        tp = p2tp.tile([128, 128], bf16, tag="tp", name="tp")
                        nc.tensor.transpose(tp[:], q_nope[:, qt, :], eye_sb[:])
                        nc.any.tensor_copy(qnT[:, qt, :], tp[:])
                        tp = p2tp.tile([128, 128], bf16, tag="tp", name="tp")
                        nc.tensor.transpose(tp[0:64, :], q_rope[:, qt, :], eye_sb[:])
                        nc.any.tensor_copy(qT[0:64, 4, qt * 128:(qt + 1) * 128],
                                           tp[0:64, :])
                    for rc in range(NKV):
                        lp = p2ps.tile([128, RPC], f32, tag="sp", name="sp")
                        nc.tensor.matmul(lp[:], wuk_h[:, rc, :], qnT[:, :, :],
                                         start=True, stop=True)
                        nc.scalar.copy(qT[:, rc, :], lp[:])

                    ol = [olps.tile([128, KVLR], f32, tag="ol", name="ol")
                          for _ in range(NQT)]
                    den = [denps.tile([128, 1], f32, tag="den", name="den")
                           for _ in range(NQT)]
                    for kt in range(NTT):
                        qs = RPC if kt < 8 else 128
                        q0 = 0 if kt < 8 else 128
                        sp = p2ps.tile([128, RPC], f32, tag="sp", name="sp")
                        for dc in range(4):
                            nc.tensor.matmul(
                                sp[:, q0:q0 + qs],
                                kT_lat[:, dc, kt * 128:(kt + 1) * 128],
                                qT[:, dc, q0:q0 + qs],
                                start=(dc == 0), stop=False)
                        nc.tensor.matmul(
                            sp[:, q0:q0 + qs],
                            kT_rope[:, kt * 128:(kt + 1) * 128],
                            qT[0:64, 4, q0:q0 + qs],
                            start=False, stop=True)
                        eT = p2d.tile([128, RPC], bf16, tag="eT", name="eT")
                        nc.scalar.activation(eT[:, q0:q0 + qs], sp[:, q0:q0 + qs],
                                             AF.Exp)
                        nc.vector.tensor_mul(eT[:, q0:q0 + qs], eT[:, q0:q0 + qs],
                                             masks_sb[:, kt, q0:q0 + qs])
                        for qt in range(NQT):
                            if qt == 0 and kt >= 8:
                                continue
                            last = 7 if qt == 0 else NTT - 1
                            nc.tensor.matmul(ol[qt][:],
                                             eT[:, qt * 128:(qt + 1) * 128],
                                             c_hat[:, kt, :],
                                             start=(kt == 0), stop=(kt == last))
                            nc.tensor.matmul(den[qt][:],
                                             eT[:, qt * 128:(qt + 1) * 128],
                                             ones_sb[:],
                                             start=(kt == 0), stop=(kt == last))
                    rinv = p2d.tile([128, NQT], f32, tag="rinv", name="rinv")
                    for qt in range(NQT):
                        nc.vector.reciprocal(rinv[:, qt:qt + 1], den[qt][:])
                    oln = p2d.tile([128, NQT, KVLR], bf16, tag="oln", name="oln")
                    for qt in range(NQT):
                        nc.vector.tensor_scalar_mul(oln[:, qt, :], ol[qt][:],
                                                    rinv[:, qt:qt + 1])
                    olT = p2d.tile([128, NKV, RPC], bf16, tag="olT", name="olT")
                    for qt in range(NQT):
                        for rc in range(NKV):
                            tp = p2tp.tile([128, 128], bf16, tag="tp", name="tp")
                            nc.tensor.transpose(tp[:],
                                                oln[:, qt, rc * 128:(rc + 1) * 128],
                                                eye_sb[:])
                            nc.any.tensor_copy(olT[:, rc, qt * 128:(qt + 1) * 128],
                                               tp[:])
                    ovp = p2ps.tile([128, RPC], f32, tag="sp", name="sp")
                    for rc in range(NKV):
                        nc.tensor.matmul(ovp[:], wuv_h[:, rc, :], olT[:, rc, :],
                                         start=(rc == 0), stop=(rc == NKV - 1))
                    nc.scalar.copy(o_vT[:, h, :], ovp[:])


        # =================== phase 3: o_proj + residual + post-norm ===========
        if MAXPH >= 3:
            with tc.tile_pool(name="p3", bufs=3) as p3, \
                 tc.tile_pool(name="p3s", bufs=1) as p3s, \
                 tc.tile_pool(name="p3d", bufs=2) as p3d:
                hn = p3s.tile([128, NQT, H], bf16, name="hn")
                with tc.tile_pool(name="p3ps", bufs=2, space="PSUM") as p3ps:
                    op_ps = [p3ps.tile([128, H], f32, tag="opps", name="opps")
                             for _ in range(NQT)]
                    for hc in range(NH):
                        oww = p3.tile([128, H], bf16, tag="oww", name="oww")
                        nc.sync.dma_start(out=oww[:], in_=ow_blk[hc])
                        for qt in range(NQT):
                            for nn in range(4):
                                nc.tensor.matmul(
                                    op_ps[qt][:, nn * 512:(nn + 1) * 512],
                                    o_vT[:, hc, qt * 128:(qt + 1) * 128],
                                    oww[:, nn * 512:(nn + 1) * 512],
                                    start=(hc == 0), stop=(hc == NH - 1))
                    ssq2 = p3s.tile([128, NQT], f32, name="ssq2")
                    for qt in range(NQT):
                        nc.vector.tensor_add(x_rows_sb[:, qt, :],
                                             x_rows_sb[:, qt, :], op_ps[qt][:])
                    for qt in range(NQT):
                        scr = p3d.tile([128, H], bf16, tag="scr3", name="scr3")
                        nc.vector.scalar_tensor_tensor(
                            scr[:], x_rows_sb[:, qt, :], 1.0, x_rows_sb[:, qt, :],
                            ALU.bypass, ALU.mult, accum_out=ssq2[:, qt:qt + 1])
                    nc.scalar.activation(ssq2[:], ssq2[:], AF.Ln, bias=eps_sb[:],
                                         scale=1.0 / H)
                    nc.scalar.activation(ssq2[:], ssq2[:], AF.Exp, scale=-0.5)
                    for qt in range(NQT):
                        nc.vector.tensor_scalar_mul(hn[:, qt, :],
                                                    x_rows_sb[:, qt, :],
                                                    ssq2[:, qt:qt + 1])
                with tc.tile_pool(name="p3tp", bufs=2, space="PSUM") as p3tp:
                    for qt in range(NQT):
                        for fc in range(NFC):
                            tp = p3tp.tile([128, 128], bf16, tag="tp", name="tp")
                            nc.tensor.transpose(tp[:],
                                                hn[:, qt, fc * 128:(fc + 1) * 128],
                                                eye_sb[:])
                            nc.any.tensor_copy(hnT[:, fc, qt * 128:(qt + 1) * 128],
                                               tp[:])


        # =================== phase 4: MLP ===================
        if MAXPH >= 4:
            with tc.tile_pool(name="p4", bufs=3) as p4, \
                 tc.tile_pool(name="p4ps", bufs=2, space="PSUM") as p4ps:
                for it in range(NIT):
                    gw = p4.tile([128, NFC, 128], bf16, tag="gw", name="gw")
                    nc.sync.dma_start(out=gw[:], in_=gu_blk[0, it])
                    uw = p4.tile([128, NFC, 128], bf16, tag="uw", name="uw")
                    nc.sync.dma_start(out=uw[:], in_=gu_blk[1, it])
                    gp = p4ps.tile([128, RPC], f32, tag="gp", name="gp")
                    up = p4ps.tile([128, RPC], f32, tag="up", name="up")
                    for fc in range(NFC):
                        nc.tensor.matmul(gp[:], gw[:, fc, :], hnT[:, fc, :],
                                         start=(fc == 0), stop=(fc == NFC - 1))
                        nc.tensor.matmul(up[:], uw[:, fc, :], hnT[:, fc, :],
                                         start=(fc == 0), stop=(fc == NFC - 1))
                    gs = p4.tile([128, RPC], bf16, tag="gs", name="gs")
                    nc.scalar.activation(gs[:], gp[:], AF.Silu)
                    nc.vector.tensor_mul(act_all[:, it, :], gs[:], up[:])
            with tc.tile_pool(name="p4b", bufs=3) as p4b, \
                 tc.tile_pool(name="p4s", bufs=2) as p4s, \
                 tc.tile_pool(name="p4bps", bufs=2, space="PSUM") as p4bps:
                o_ps = [p4bps.tile([128, H], f32, tag="ops", name="ops")
                        for _ in range(NQT)]
                for it in range(NIT):
                    dw = p4b.tile([128, H], bf16, tag="dw", name="dw")
                    nc.sync.dma_start(out=dw[:], in_=dw_blk[it])
                    for qt in range(NQT):
                        for nn in range(4):
                            nc.tensor.matmul(
                                o_ps[qt][:, nn * 512:(nn + 1) * 512],
                                act_all[:, it, qt * 128:(qt + 1) * 128],
                                dw[:, nn * 512:(nn + 1) * 512],
                                start=(it == 0), stop=(it == NIT - 1))
                for qt in range(NQT):
                    fin = p4s.tile([128, H], f32, tag="fin", name="fin")
                    nc.vector.tensor_add(fin[:], x_rows_sb[:, qt, :], o_ps[qt][:])
                    nc.sync.dma_start(out=out_rows[qt], in_=fin[:])

        if MAXPH < 4:
            with tc.tile_pool(name="pex", bufs=2) as pex:
                for qt in range(NQT):
                    fin = pex.tile([128, H], f32, tag="finx", name="finx")
                    nc.vector.tensor_copy(fin[:], x_rows_sb[:, qt, :])
                    nc.sync.dma_start(out=out_rows[qt], in_=fin[:])
    nc.compile()
    return nc


def _host_prep(inputs):
    f32 = np.float32
    bf = bfloat16
    x = np.asarray(inputs["hidden_states"], f32)
    pos = np.asarray(inputs["positions"]).astype(f32)

    lnw_in = np.asarray(inputs["input_ln_w"], f32)
    q_a_w = np.asarray(inputs["q_a_w"], f32) * lnw_in[:, None]
    kv_a_w = np.asarray(inputs["kv_a_w"], f32) * lnw_in[:, None]
    q_b_w = (np.asarray(inputs["q_b_w"], f32)
             * np.asarray(inputs["q_a_ln_w"], f32)[:, None])
    kvln = np.asarray(inputs["kv_a_ln_w"], f32)
    w_uk = np.asarray(inputs["w_uk"], f32) * kvln[:, None, None]
    w_uv = np.asarray(inputs["w_uv"], f32) * kvln[:, None, None]
    o_w = np.asarray(inputs["o_w"], f32)
    pln = np.asarray(inputs["post_ln_w"], f32)
    gate_w = np.asarray(inputs["gate_w"], f32) * pln[:, None]
    up_w = np.asarray(inputs["up_w"], f32) * pln[:, None]
    down_w = np.asarray(inputs["down_w"], f32)

    xT = np.ascontiguousarray(x.T)
    inv_freq = 1.0 / (THETA ** (np.arange(0, DR, 2, dtype=f32) / DR))
    ang = pos[:, None] * inv_freq
    cos_t = np.cos(ang).astype(f32)
    sin_t = np.sin(ang).astype(f32)

    gu = np.zeros((2, IPAD, H), f32)
    gu[0, :INTER] = gate_w.T
    gu[1, :INTER] = up_w.T

    rep = {
        "xstat": np.ascontiguousarray(x.reshape(NTT, 128, H).astype(bf)),
        "xT_blk": np.ascontiguousarray(
            xT.astype(bf).reshape(NFC, 128, NTT, 128).transpose(2, 1, 0, 3)),
        "qa_blk": np.ascontiguousarray(q_a_w.astype(bf).reshape(NFC, 128, QLR)),
        # qb_blk[h, rc, p, d] = q_b_w[rc*128+p, h*192+d]
        "qb_blk": np.ascontiguousarray(
            q_b_w.astype(bf).reshape(NRC, 128, NH, QH).transpose(2, 0, 1, 3)),
        "kva_blk": np.ascontiguousarray(
            kv_a_w.astype(bf).reshape(NFC, 128, KVLR + DR)),
        # wuk[h, d, rc, rr] = w_uk[rc*128+rr, h, d]
        "wuk": np.ascontiguousarray(
            w_uk.transpose(1, 2, 0).reshape(NH, 128, NKV, 128).astype(bf)),
        # wuv[h, p, rc, dv] = w_uv[rc*128+p, h, dv]
        "wuv": np.ascontiguousarray(
            w_uv.transpose(1, 0, 2).reshape(NH, NKV, 128, DV)
            .transpose(0, 2, 1, 3).astype(bf)),
        "ow_blk": np.ascontiguousarray(o_w.astype(bf).reshape(NH, 128, H)),
        "gu_blk": np.ascontiguousarray(
            gu.reshape(2, NIT, 128, NFC, 128).transpose(0, 1, 4, 3, 2)
            .astype(bf)),
        "dw_blk": np.ascontiguousarray(
            np.concatenate([down_w, np.zeros((IPAD - INTER, H), f32)], 0)
            .astype(bf).reshape(NIT, 128, H)),
        "cosk": np.ascontiguousarray(
            cos_t.reshape(NTT, 128, DR // 2).transpose(1, 0, 2)),
        "sink": np.ascontiguousarray(
            sin_t.reshape(NTT, 128, DR // 2).transpose(1, 0, 2)),
        "eye": np.eye(128, dtype=bf),
        "ones": np.ones((128, 1), bf),
    }

    per_core = []
    for c in range(NCORES):
        rows = np.arange(c, T, NCORES)
        m = dict(rep)
        m["x_rows"] = np.ascontiguousarray(x[rows].reshape(NQT, 128, H))
        m["xTc"] = np.ascontiguousarray(
            xT[:, rows].astype(bf).reshape(NFC, 128, RPC))
        m["cosq"] = np.ascontiguousarray(cos_t[rows].reshape(NQT, 128, DR // 2))
        m["sinq"] = np.ascontiguousarray(sin_t[rows].reshape(NQT, 128, DR // 2))
        mask = np.zeros((NTT, 128, RPC), f32)
        kpos = np.arange(128)
        for kt in range(NTT):
            gk = kt * 128 + kpos
            mask[kt] = (gk[:, None] <= rows[None, :]).astype(f32)
        m["masks"] = mask.astype(bf)
        per_core.append(m)
    return per_core


def kernel(**inputs):
    from concourse import bass_utils

    if "nc" not in _CACHE:
        _CACHE["nc"] = _build_module()
    nc = _CACHE["nc"]

    import os
    in_maps = _host_prep(inputs)
    trace = bool(os.environ.get("BASS_KERNEL_TRACE"))
    res = bass_utils.run_bass_kernel_spmd(nc, in_maps,
                                          core_ids=list(range(NCORES)),
                                          trace=trace)
    if trace and res.exec_time_ns is not None:
        print(f"HW exec time: {res.exec_time_ns} ns")
        _CACHE["last_result"] = res
    out = np.zeros((T, H), np.float32)
    for c in range(NCORES):
        rows = np.arange(c, T, NCORES)
        out[rows] = res.results[c]["out_rows"].reshape(RPC, H)
    return out

